# revision 3
# baseline (speedup 1.0000x reference)
"""Binarized MLP (784 -> 1024 -> 512 -> 256 -> 10, sign(W) weights) on 8 TRN2 cores.

Strategy: pure data parallel. The batch (16384) is split into 8 shards of
2048 rows, the small binarized weights are replicated. Host-side prep:
  - weights are binarized (sign), transposed to [in, out], cast to bf16
    (+-1 is exact in bf16),
  - each x shard is transposed to feature-major [784, 2048], cast to bf16,
    and zero-padded to 896 (=7*128) rows so the contraction dim tiles evenly.
On device everything stays in SBUF (about 120KB/partition); each layer is a
tiled matmul with fp32 PSUM accumulation and a fused bias+ReLU (ScalarE)
that also casts back to bf16. Output is produced feature-major [10, 2048]
fp32 and transposed back on the host.
"""

from contextlib import ExitStack

import ml_dtypes
import numpy as np

import concourse.bass as bass
import concourse.mybir as mybir
import concourse.tile as tile
from concourse import bacc
from concourse.bass_utils import run_bass_kernel_spmd

N_CORES = 8
B_FULL = 16384
B = B_FULL // N_CORES  # 2048 rows per core
D_IN = 784
K1 = 896  # 784 zero-padded to 7*128
NT = 512  # batch tile (one PSUM bank of fp32)
NB = B // NT  # 4

BF16 = mybir.dt.bfloat16
F32 = mybir.dt.float32
npbf16 = ml_dtypes.bfloat16

# (K, O) per layer, K padded to a multiple of 128
LAYER_DIMS = [(K1, 1024), (1024, 512), (512, 256), (256, 10)]

_prog_cache = {}


def _build_program():
    nc = bacc.Bacc("TRN2", target_bir_lowering=False, debug=False)

    xt_d = nc.dram_tensor("xt", [K1, B], BF16, kind="ExternalInput")
    w_d = [
        nc.dram_tensor(f"w{i + 1}t", [k, o], BF16, kind="ExternalInput")
        for i, (k, o) in enumerate(LAYER_DIMS)
    ]
    # biases laid out [partition, o_tile] (o = o_tile*128 + partition)
    b_d = [
        nc.dram_tensor(f"b{i + 1}r", [min(o, 128), max(1, o // 128)], F32, kind="ExternalInput")
        for i, (_, o) in enumerate(LAYER_DIMS)
    ]
    out_d = nc.dram_tensor("out_t", [10, B], F32, kind="ExternalOutput")

    with tile.TileContext(nc) as tc, ExitStack() as ctx:
        res = ctx.enter_context(tc.tile_pool(name="resident", bufs=1))
        psum = ctx.enter_context(tc.tile_pool(name="psum", bufs=8, space="PSUM"))

        relu = mybir.ActivationFunctionType.Relu
        ident = mybir.ActivationFunctionType.Identity

        # ---- load weights + biases + x (all resident in SBUF) ----
        w_sb = []  # per layer: list of [128, O] bf16 tiles (one per k-tile)
        b_sb = []  # per layer: [128, n_o] f32
        for li, (k, o) in enumerate(LAYER_DIMS):
            nk = k // 128
            tiles = []
            for kt in range(nk):
                wt = res.tile([128, o], BF16, tag=f"w{li}_{kt}", name=f"w{li}_{kt}")
                nc.sync.dma_start(wt[:, :], w_d[li][kt * 128 : (kt + 1) * 128, :])
                tiles.append(wt)
            w_sb.append(tiles)
            bt = res.tile(
                [min(o, 128), max(1, o // 128)], F32, tag=f"b{li}", name=f"b{li}"
            )
            nc.sync.dma_start(bt[:, :], b_d[li][:, :])
            b_sb.append(bt)

        x_sb = []
        for kt in range(K1 // 128):
            xtile = res.tile([128, B], BF16, tag=f"x_{kt}", name=f"x_{kt}")
            nc.sync.dma_start(xtile[:, :], xt_d[kt * 128 : (kt + 1) * 128, :])
            x_sb.append(xtile)

        # ---- layers ----
        acts = x_sb
        for li, (k, o) in enumerate(LAYER_DIMS):
            nk = k // 128
            last = li == len(LAYER_DIMS) - 1
            if last:
                h_tiles = [res.tile([o, B], F32, tag="hout", name="hout")]
            else:
                h_tiles = [
                    res.tile([128, B], BF16, tag=f"h{li}_{ot}", name=f"h{li}_{ot}")
                    for ot in range(o // 128)
                ]
            n_o = max(1, o // 128)
            po = min(o, 128)  # psum partitions (10 for the last layer)
            for ot in range(n_o):
                for n in range(NB):
                    ps = psum.tile([po, NT], F32, tag="ps", name="ps")
                    for kt in range(nk):
                        nc.tensor.matmul(
                            ps[:, :],
                            w_sb[li][kt][:, ot * po : ot * po + po],
                            acts[kt][:, n * NT : (n + 1) * NT],
                            start=(kt == 0),
                            stop=(kt == nk - 1),
                        )
                    nc.scalar.activation(
                        h_tiles[ot][:, n * NT : (n + 1) * NT] if not last else h_tiles[0][:, n * NT : (n + 1) * NT],
                        ps[:, :],
                        ident if last else relu,
                        bias=b_sb[li][:, ot : ot + 1],
                    )
            acts = h_tiles

        nc.sync.dma_start(out_d[:, :], acts[0][:, :])

    nc.compile()
    return nc


def _get_program():
    if "nc" not in _prog_cache:
        _prog_cache["nc"] = _build_program()
    return _prog_cache["nc"]


def _prep_shared(w1, b1, w2, b2, w3, b3, w4, b4):
    ws = [np.asarray(w, np.float32) for w in (w1, w2, w3, w4)]
    bs = [np.asarray(b, np.float32) for b in (b1, b2, b3, b4)]
    shared = {}
    for i, ((k, o), w) in enumerate(zip(LAYER_DIMS, ws)):
        wt = np.zeros((k, o), dtype=npbf16)
        wt[: w.shape[1], :] = np.sign(w).T.astype(npbf16)
        shared[f"w{i + 1}t"] = wt
        b = bs[i]
        if o >= 128:
            br = np.ascontiguousarray(b.reshape(o // 128, 128).T)
        else:
            br = np.ascontiguousarray(b.reshape(o, 1))
        shared[f"b{i + 1}r"] = br
    return shared


def _run(inputs, trace=False, tmpdir=None):
    x = np.asarray(inputs["x"], np.float32)
    assert x.shape == (B_FULL, D_IN), x.shape
    nc = _get_program()
    shared = _prep_shared(
        inputs["w1"], inputs["b1"], inputs["w2"], inputs["b2"],
        inputs["w3"], inputs["b3"], inputs["w4"], inputs["b4"],
    )
    in_maps = []
    for c in range(N_CORES):
        xs = x[c * B : (c + 1) * B]  # [B, 784]
        xt = np.zeros((K1, B), dtype=npbf16)
        xt[:D_IN, :] = xs.T.astype(npbf16)
        in_maps.append({"xt": xt, **shared})
    res = run_bass_kernel_spmd(
        nc, in_maps, core_ids=list(range(N_CORES)), trace=trace, tmpdir=tmpdir
    )
    out = np.concatenate(
        [np.ascontiguousarray(r["out_t"].T) for r in res.results], axis=0
    )
    return out.astype(np.float32), res


def kernel(**inputs):
    out, _ = _run(inputs, trace=False)
    return out


# revision 8
# speedup vs baseline: 1.0324x; 1.0324x over previous
"""Binarized MLP (784 -> 1024 -> 512 -> 256 -> 10, sign(W) weights) on 8 TRN2 cores.

Strategy: pure data parallel. The batch (16384) is split into 8 shards of
2048 rows, the small binarized weights are replicated. Host-side prep:
  - weights are binarized (sign), transposed to [in, out], cast to bf16
    (+-1 is exact in bf16),
  - each x shard is transposed to feature-major [784, 2048], cast to bf16,
    and zero-padded to 896 (=7*128) rows so the contraction dim tiles evenly.
On device everything stays in SBUF (about 120KB/partition); each layer is a
tiled matmul with fp32 PSUM accumulation and a fused bias+ReLU (ScalarE)
that also casts back to bf16. Output is produced feature-major [10, 2048]
fp32 and transposed back on the host.
"""

from contextlib import ExitStack

import ml_dtypes
import numpy as np

import concourse.bass as bass
import concourse.mybir as mybir
import concourse.tile as tile
from concourse import bacc
from concourse.bass_utils import run_bass_kernel_spmd

N_CORES = 8
B_FULL = 16384
B = B_FULL // N_CORES  # 2048 rows per core
D_IN = 784
K1 = D_IN  # ragged: 6 full k-tiles + one K=16 tile
NT = 512  # batch tile (one PSUM bank of fp32)
NB = B // NT  # 4

BF16 = mybir.dt.bfloat16
F32 = mybir.dt.float32
npbf16 = ml_dtypes.bfloat16

# (K, O) per layer
LAYER_DIMS = [(K1, 1024), (1024, 512), (512, 256), (256, 10)]


def _ktiles(k):
    """Split contraction dim into (start, size) tiles of <=128."""
    return [(s, min(128, k - s)) for s in range(0, k, 128)]

_prog_cache = {}


def _build_program():
    nc = bacc.Bacc("TRN2", target_bir_lowering=False, debug=False)

    xt_d = nc.dram_tensor("xt", [K1, B], BF16, kind="ExternalInput")
    w_d = [
        nc.dram_tensor(f"w{i + 1}t", [k, o], BF16, kind="ExternalInput")
        for i, (k, o) in enumerate(LAYER_DIMS)
    ]
    # biases laid out [partition, o_tile] (o = o_tile*128 + partition)
    b_d = [
        nc.dram_tensor(f"b{i + 1}r", [min(o, 128), max(1, o // 128)], F32, kind="ExternalInput")
        for i, (_, o) in enumerate(LAYER_DIMS)
    ]
    out_d = nc.dram_tensor("out_t", [10, B], F32, kind="ExternalOutput")

    with tile.TileContext(nc) as tc, ExitStack() as ctx:
        res = ctx.enter_context(tc.tile_pool(name="resident", bufs=1))
        psum = ctx.enter_context(tc.tile_pool(name="psum", bufs=8, space="PSUM"))

        relu = mybir.ActivationFunctionType.Relu
        ident = mybir.ActivationFunctionType.Identity

        # ---- loads, in consumption order: (x[k], w1[k]) pairs gate the
        # first layer, so they go first; later layers' weights follow.
        x_sb = []
        w_sb = [[] for _ in LAYER_DIMS]
        for kt, (ks, kn) in enumerate(_ktiles(K1)):
            xtile = res.tile([kn, B], BF16, tag=f"x_{kt}", name=f"x_{kt}")
            nc.sync.dma_start(xtile[:, :], xt_d[ks : ks + kn, :])
            x_sb.append(xtile)
            wt = res.tile([kn, 1024], BF16, tag=f"w0_{kt}", name=f"w0_{kt}")
            nc.sync.dma_start(wt[:, :], w_d[0][ks : ks + kn, :])
            w_sb[0].append(wt)
        for li, (k, o) in enumerate(LAYER_DIMS):
            if li == 0:
                continue
            for kt, (ks, kn) in enumerate(_ktiles(k)):
                wt = res.tile([kn, o], BF16, tag=f"w{li}_{kt}", name=f"w{li}_{kt}")
                nc.sync.dma_start(wt[:, :], w_d[li][ks : ks + kn, :])
                w_sb[li].append(wt)
        b_sb = []
        for li, (_, o) in enumerate(LAYER_DIMS):
            bt = res.tile(
                [min(o, 128), max(1, o // 128)], F32, tag=f"b{li}", name=f"b{li}"
            )
            nc.sync.dma_start(bt[:, :], b_d[li][:, :])
            b_sb.append(bt)

        # ---- layers ----
        acts = x_sb
        for li, (k, o) in enumerate(LAYER_DIMS):
            nk = len(_ktiles(k))
            last = li == len(LAYER_DIMS) - 1
            if last:
                h_tiles = [res.tile([o, B], F32, tag="hout", name="hout")]
            else:
                h_tiles = [
                    res.tile([128, B], BF16, tag=f"h{li}_{ot}", name=f"h{li}_{ot}")
                    for ot in range(o // 128)
                ]
            n_o = max(1, o // 128)
            po = min(o, 128)  # psum partitions (10 for the last layer)
            for ot in range(n_o):
                for n in range(NB):
                    ps = psum.tile([po, NT], F32, tag="ps", name="ps")
                    for kt in range(nk):
                        nc.tensor.matmul(
                            ps[:, :],
                            w_sb[li][kt][:, ot * po : ot * po + po],
                            acts[kt][:, n * NT : (n + 1) * NT],
                            start=(kt == 0),
                            stop=(kt == nk - 1),
                        )
                    nc.scalar.activation(
                        h_tiles[ot][:, n * NT : (n + 1) * NT] if not last else h_tiles[0][:, n * NT : (n + 1) * NT],
                        ps[:, :],
                        ident if last else relu,
                        bias=b_sb[li][:, ot : ot + 1],
                    )
            acts = h_tiles

        nc.sync.dma_start(out_d[:, :], acts[0][:, :])

    nc.compile()
    return nc


def _get_program():
    if "nc" not in _prog_cache:
        _prog_cache["nc"] = _build_program()
    return _prog_cache["nc"]


def _prep_shared(w1, b1, w2, b2, w3, b3, w4, b4):
    ws = [np.asarray(w, np.float32) for w in (w1, w2, w3, w4)]
    bs = [np.asarray(b, np.float32) for b in (b1, b2, b3, b4)]
    shared = {}
    for i, ((k, o), w) in enumerate(zip(LAYER_DIMS, ws)):
        wt = np.ascontiguousarray(np.sign(w).T.astype(npbf16))
        assert wt.shape == (k, o)
        shared[f"w{i + 1}t"] = wt
        b = bs[i]
        if o >= 128:
            br = np.ascontiguousarray(b.reshape(o // 128, 128).T)
        else:
            br = np.ascontiguousarray(b.reshape(o, 1))
        shared[f"b{i + 1}r"] = br
    return shared


def _run(inputs, trace=False, tmpdir=None):
    x = np.asarray(inputs["x"], np.float32)
    assert x.shape == (B_FULL, D_IN), x.shape
    nc = _get_program()
    shared = _prep_shared(
        inputs["w1"], inputs["b1"], inputs["w2"], inputs["b2"],
        inputs["w3"], inputs["b3"], inputs["w4"], inputs["b4"],
    )
    in_maps = []
    for c in range(N_CORES):
        xs = x[c * B : (c + 1) * B]  # [B, 784]
        xt = np.ascontiguousarray(xs.T.astype(npbf16))  # [784, B]
        in_maps.append({"xt": xt, **shared})
    res = run_bass_kernel_spmd(
        nc, in_maps, core_ids=list(range(N_CORES)), trace=trace, tmpdir=tmpdir
    )
    out = np.concatenate(
        [np.ascontiguousarray(r["out_t"].T) for r in res.results], axis=0
    )
    return out.astype(np.float32), res


def kernel(**inputs):
    out, _ = _run(inputs, trace=False)
    return out
